# revision 45
# baseline (speedup 1.0000x reference)
"""Trainium2 Bass kernel for nn_BDH_39127152067244 (dense_transformer).

Sharding: 8 cores = (b, h) pairs — b = core // 4, h = core % 4. Each core
computes its head's share of every layer; the only cross-core communication
is an AllReduce of the per-head yMLP partial [T, D] once per layer
(replica groups {0..3} and {4..7}), split into two n-halves so the first
AllReduce overlaps the second half of the E/F compute.

Layout tricks:
  - The N axis (8192) is deinterleaved on the host (even n first, odd n
    second). Even/odd pairs share a rope frequency, so the cos/sin tables
    are stored at HALF size ([P, 32, T]) and the rotate-half sign becomes
    subtract (first half) vs add (second half).
  - x_sparse is computed directly in transposed [N, T] layout; the Gram
    matmul (scores) is fused into the A phase: each rope'd group feeds its
    Gram contributions immediately, so the PE never drains between phases.
  - scores: symmetric Gram, strict-lower mask in [t, s] == strict-upper in
    [s, t]; only the needed triangle of tiles is computed.
  - E (y_sparse) and F (decoder matmul) are fused per weight tile; F
    accumulates the two n-halves into separate PSUM half-banks so the first
    half's AllReduce (f32) is fired mid-phase and hidden under compute.
  - All matmuls in bf16 with f32 PSUM accumulation; LayerNorm chains use
    multi-group bn_stats + scalar Rsqrt; residual stream stays f32.
"""

import math
import sys
from contextlib import ExitStack

import numpy as np
import ml_dtypes

sys.path.insert(0, "/opt/trn_rl_repo")

import concourse.bass as bass  # noqa: E402
import concourse.bacc as bacc  # noqa: E402
import concourse.mybir as mybir  # noqa: E402
import concourse.tile as tile  # noqa: E402
from concourse.bass import ds  # noqa: E402
from concourse.bass_utils import run_bass_kernel_spmd  # noqa: E402
from concourse.masks import make_identity  # noqa: E402

BF16 = ml_dtypes.bfloat16
BF = mybir.dt.bfloat16
FP32 = mybir.dt.float32
AF = mybir.ActivationFunctionType
ALU = mybir.AluOpType

# Problem constants (hardcoded per the harness contract).
N_LAYER = 6
D = 256
NH = 4
N = 8192
HALF = N // 2
VOCAB = 256
B, T = 2, 512
THETA = 2.0**16
EPS = 1e-5

P = 128          # partitions
NT = N // P      # 64 n-tiles
NT2 = NT // 2    # 32 table tiles (half-size rope tables)
G4 = 4           # n-tiles per rope/qx group
NG = NT // G4    # 16 groups
VG = 8           # n-tiles per V tile
NVG = NT // VG   # 8 V tiles
TC = T // P      # 4 t-chunks
DT = D // P      # 2 d-tiles
N_CORES = 8
REPL = [[0, 1, 2, 3], [4, 5, 6, 7]]

_CACHE: dict = {}


def _build_bass():
    nc = bacc.Bacc("TRN2", num_devices=N_CORES)

    x0_d = nc.dram_tensor("x0", [P, TC, D], FP32, kind="ExternalInput")
    x0bf_d = nc.dram_tensor("x0bf", [P, TC, D], BF, kind="ExternalInput")
    x0T_d = nc.dram_tensor("x0T", [P, DT, T], BF, kind="ExternalInput")
    enc_d = nc.dram_tensor("enc", [DT, P, NT, P], BF, kind="ExternalInput")
    encv_d = nc.dram_tensor("encv", [DT, P, NT, P], BF, kind="ExternalInput")
    dec_d = nc.dram_tensor("dec", [P, NT, D], BF, kind="ExternalInput")
    cos_d = nc.dram_tensor("cosb", [P, NT2, T], BF, kind="ExternalInput")
    sin_d = nc.dram_tensor("sinb", [P, NT2, T], BF, kind="ExternalInput")
    mask_d = nc.dram_tensor("maskb", [P, TC, T], BF, kind="ExternalInput")
    lm_d = nc.dram_tensor("lm", [P, DT, VOCAB], BF, kind="ExternalInput")
    out_d = nc.dram_tensor("logits", [P, TC, VOCAB], FP32, kind="ExternalOutput")

    with tile.TileContext(nc) as tc, ExitStack() as ctx:
        sb = ctx.enter_context(tc.tile_pool(name="sb", bufs=1))
        vpool = ctx.enter_context(tc.tile_pool(name="vpool", bufs=NVG))
        qrpool = ctx.enter_context(tc.tile_pool(name="qrpool", bufs=3))
        xypool = ctx.enter_context(tc.tile_pool(name="xypool", bufs=4))
        yspool = ctx.enter_context(tc.tile_pool(name="yspool", bufs=3))
        rppool = ctx.enter_context(tc.tile_pool(name="rppool", bufs=2))
        tabpool = ctx.enter_context(tc.tile_pool(name="tabpool", bufs=2))
        wpool = ctx.enter_context(tc.tile_pool(name="wpool", bufs=3))
        mixpool = ctx.enter_context(tc.tile_pool(name="mixpool", bufs=1))
        statpool = ctx.enter_context(tc.tile_pool(name="statpool", bufs=4))
        xpool = ctx.enter_context(tc.tile_pool(name="xpool", bufs=2))
        apsum = ctx.enter_context(tc.tile_pool(name="apsum", bufs=2, space="PSUM"))
        cpsum = ctx.enter_context(tc.tile_pool(name="cpsum", bufs=1, space="PSUM"))
        drm = ctx.enter_context(tc.tile_pool(name="drm", bufs=2, space="DRAM"))

        x_T = xpool.tile([P, DT, T], BF, tag="xT", name="x_T0")
        nc.sync.dma_start(out=x_T, in_=x0T_d[:])
        x_bf = xpool.tile([P, TC, D], BF, tag="xbf", name="x_bf0")
        nc.sync.dma_start(out=x_bf, in_=x0bf_d[:])
        x_f = xpool.tile([P, TC, D], FP32, tag="xf", name="x_f0")
        nc.sync.dma_start(out=x_f, in_=x0_d[:])

        ident = sb.tile([P, P], BF, name="ident")
        make_identity(nc, ident)
        epst = sb.tile([P, 1], FP32, name="epst")
        nc.vector.memset(epst, EPS)
        maskt = sb.tile([P, TC, T], BF, name="maskt")
        nc.sync.dma_start(out=maskt, in_=mask_d[:])
        lmt = sb.tile([P, DT, VOCAB], BF, name="lmt")
        nc.sync.dma_start(out=lmt, in_=lm_d[:])

        # Warm up the collective path before layer 0 needs it.
        wsb = sb.tile([P, 16], FP32, name="wsb")
        nc.vector.memset(wsb, 0.0)
        warm_in = drm.tile([P, 16], FP32, tag="win", name="warm_in")
        warm_out = drm.tile([P, 16], FP32, tag="wout", name="warm_out")
        nc.sync.dma_start(out=warm_in[:], in_=wsb)
        nc.gpsimd.collective_compute(
            "AllReduce", ALU.add, replica_groups=REPL,
            ins=[warm_in[:]], outs=[warm_out[:]],
        )

        def emit_layer(l, x_f, x_bf, x_T):
            # ============ fused A + rope + Gram (C) =======================
            V = [None] * NVG
            gps = cpsum.tile([P, TC, T], FP32, tag="mm", name=f"gps{l}")

            def emit_A(vg):
                encg = wpool.tile([P, DT, VG, P], BF, tag="w", name=f"enc{l}_{vg}")
                nc.sync.dma_start(
                    out=encg,
                    in_=enc_d[:, :, ds(vg * VG, VG), :].rearrange(
                        "dt p nt n -> p dt nt n"
                    ),
                )
                vt = vpool.tile([P, VG, T], BF, tag="v", name=f"v{l}_{vg}")
                V[vg] = vt
                for q in range(VG // 2):
                    ps = apsum.tile(
                        [P, 2, T], FP32, tag="quad", name=f"aps{l}_{vg}_{q}"
                    )
                    for i in range(2):
                        for dt_ in range(DT):
                            nc.tensor.matmul(
                                ps[:, i, :],
                                lhsT=encg[:, dt_, q * 2 + i, :],
                                rhs=x_T[:, dt_, :],
                                start=(dt_ == 0),
                                stop=(dt_ == DT - 1),
                            )
                    nc.scalar.activation(
                        out=vt[:, ds(q * 2, 2), :], in_=ps, func=AF.Relu
                    )

            def emit_rope(g, cosg, sing, off4):
                # qr = V[g] * cos -/+ V[g^8] * |sin|  (minus for g<8)
                vg_, off = divmod(g * G4, VG)
                pvg_, poff = divmod((g ^ (NG // 2)) * G4, VG)
                p2 = rppool.tile([P, G4, T], BF, tag="rp2", name=f"rq{l}_{g}")
                nc.vector.tensor_mul(
                    p2, V[pvg_][:, ds(poff, G4), :], sing[:, ds(off4, G4), :]
                )
                qr = qrpool.tile([P, G4, T], BF, tag="qr", name=f"qr{l}_{g}")
                nc.vector.tensor_mul(
                    qr, V[vg_][:, ds(off, G4), :], cosg[:, ds(off4, G4), :]
                )
                if g < NG // 2:
                    nc.vector.tensor_sub(qr, qr, p2)
                else:
                    nc.vector.tensor_add(qr, qr, p2)
                return qr

            def emit_pair_ropes_C(pair):
                cosg = tabpool.tile([P, VG, T], BF, tag="cos", name=f"cos{l}_{pair}")
                nc.sync.dma_start(out=cosg, in_=cos_d[:, ds(pair * VG, VG), :])
                sing = tabpool.tile([P, VG, T], BF, tag="sin", name=f"sin{l}_{pair}")
                nc.sync.dma_start(out=sing, in_=sin_d[:, ds(pair * VG, VG), :])
                gcnt = pair * 4
                for gl in (2 * pair, 2 * pair + 1):
                    off4 = (gl & 1) * G4
                    for g in (gl, gl + NG // 2):
                        qr = emit_rope(g, cosg, sing, off4)
                        for i in range(G4):
                            first = gcnt == 0 and i == 0
                            last = gcnt == NG - 1 and i == G4 - 1
                            for j in range(TC):
                                nc.tensor.matmul(
                                    gps[:, j, : T - j * P],
                                    lhsT=qr[:, i, ds(j * P, P)],
                                    rhs=qr[:, i, ds(j * P, T - j * P)],
                                    start=first,
                                    stop=last,
                                )
                        gcnt += 1

            # C trails A by one pair so the rope chain (vector) has a full
            # extra pair-time to complete before its Gram matmuls are due.
            for pair in range(NVG // 2):
                emit_A(pair)
                emit_A(pair + NVG // 2)
                if pair > 0:
                    emit_pair_ropes_C(pair - 1)
            emit_pair_ropes_C(NVG // 2 - 1)

            # ============ D: yKV = M^T @ x, pipelined LN ==================
            st = mixpool.tile([P, TC, T], BF, tag="st", name=f"st{l}")
            for i in range(TC):
                # only the diagonal 128-block needs masking; the off-diagonal
                # region is strictly upper (mask == 1) — plain copy on scalar
                nc.vector.tensor_mul(
                    st[:, i, ds(i * P, P)],
                    gps[:, i, :P],
                    maskt[:, i, ds(i * P, P)],
                )
                if i < TC - 1:
                    nc.scalar.copy(
                        out=st[:, i, ds((i + 1) * P, T - (i + 1) * P)],
                        in_=gps[:, i, P : T - i * P],
                    )
            dps = cpsum.tile([P, TC, T], FP32, tag="mm", name=f"dps{l}")
            statd = statpool.tile([P, TC, 6], FP32, tag="sd", name=f"sd{l}")
            mvd = statpool.tile([P, TC, 2], FP32, tag="md", name=f"md{l}")
            rsd = statpool.tile([P, TC, 1], FP32, tag="rd", name=f"rd{l}")
            yln = mixpool.tile([P, TC, D], BF, tag="yln", name=f"yln{l}")
            for jp in range(TC):
                for i in range(jp + 1):
                    nc.tensor.matmul(
                        dps[:, jp, :D],
                        lhsT=st[:, i, ds(jp * P, P)],
                        rhs=x_bf[:, i, :],
                        start=(i == 0),
                        stop=(i == jp),
                    )
                nc.vector.bn_stats(out=statd[:, jp, :], in_=dps[:, jp, :D])
                nc.vector.bn_aggr(out=mvd[:, jp, :], in_=statd[:, jp, :])
            nc.scalar.activation(
                out=rsd, in_=mvd[:, :, 1:2], func=AF.Sqrt, bias=epst
            )
            nc.vector.reciprocal(rsd, rsd)
            for jp in range(TC):
                nc.vector.tensor_scalar(
                    out=yln[:, jp, :],
                    in0=dps[:, jp, :D],
                    scalar1=mvd[:, jp, 0:1],
                    scalar2=rsd[:, jp, :],
                    op0=ALU.subtract,
                    op1=ALU.mult,
                )
            ylnT = mixpool.tile([P, DT, T], BF, tag="ylnT", name=f"ylnT{l}")
            for dt_ in range(DT):
                tp = apsum.tile([P, TC, P], BF, tag="quad", name=f"ytp{l}_{dt_}")
                for jp in range(TC):
                    nc.tensor.transpose(
                        tp[:, jp, :], yln[:, jp, ds(dt_ * P, P)], ident
                    )
                out_ap = ylnT[:, dt_, :].rearrange("p (a b) -> p a b", a=TC)
                if dt_ == 0:
                    nc.scalar.copy(out=out_ap, in_=tp)
                else:
                    nc.vector.tensor_copy(out=out_ap, in_=tp)

            # ============ fused E (gated y_sparse) + F (decoder) ==========
            # fps bank m: n-half A partial in cols [0,D), half B in [D,2D)
            fps = cpsum.tile([P, TC, T], FP32, tag="mm", name=f"fps{l}")
            ymA = mixpool.tile([P, TC, D], BF, tag="ymA", name=f"ymA{l}")
            ymB = mixpool.tile([P, TC, D], BF, tag="ymB", name=f"ymB{l}")
            ccA_in = drm.tile([P, TC, D], BF, tag="ccAi", name=f"ccAi{l}")
            ccA_out = drm.tile([P, TC, D], BF, tag="ccAo", name=f"ccAo{l}")
            ccB_in = drm.tile([P, TC, D], BF, tag="ccBi", name=f"ccBi{l}")
            ccB_out = drm.tile([P, TC, D], BF, tag="ccBo", name=f"ccBo{l}")
            ymr = mixpool.tile([P, TC, D], BF, tag="ymr", name=f"ymr{l}")
            ymr2 = mixpool.tile([P, TC, D], BF, tag="ymr2", name=f"ym2{l}")

            for vg in range(NVG):
                evg = wpool.tile([P, DT, VG, P], BF, tag="w", name=f"ev{l}_{vg}")
                nc.sync.dma_start(
                    out=evg,
                    in_=encv_d[:, :, ds(vg * VG, VG), :].rearrange(
                        "dt p nt n -> p dt nt n"
                    ),
                )
                decg = wpool.tile([P, VG, D], BF, tag="dec", name=f"dec{l}_{vg}")
                nc.sync.dma_start(out=decg, in_=dec_d[:, ds(vg * VG, VG), :])
                half = vg // (NVG // 2)
                vgh = vg % (NVG // 2)

                def emit_E(q):
                    ps = apsum.tile(
                        [P, 2, T], FP32, tag="quad", name=f"eps{l}_{vg}_{q}"
                    )
                    for i in range(2):
                        for dt_ in range(DT):
                            nc.tensor.matmul(
                                ps[:, i, :],
                                lhsT=evg[:, dt_, q * 2 + i, :],
                                rhs=ylnT[:, dt_, :],
                                start=(dt_ == 0),
                                stop=(dt_ == DT - 1),
                            )
                    ys = yspool.tile([P, 2, T], BF, tag="ys", name=f"ys{l}_{vg}_{q}")
                    nc.scalar.activation(out=ys, in_=ps, func=AF.Relu)
                    xy = xypool.tile([P, 2, T], BF, tag="xy", name=f"xy{l}_{vg}_{q}")
                    nc.vector.tensor_mul(xy, ys, V[vg][:, ds(q * 2, 2), :])
                    return xy

                def emit_F(q, xy):
                    for i in range(2):
                        kh = vgh * VG + q * 2 + i
                        for m in range(TC):
                            nc.tensor.matmul(
                                fps[:, m, ds(half * D, D)],
                                lhsT=xy[:, i, ds(m * P, P)],
                                rhs=decg[:, q * 2 + i, :],
                                start=(kh == 0),
                                stop=(kh == NT // 2 - 1),
                            )

                # one-quad software pipeline: F trails E so the relu+gate
                # chain of quad q completes before its F matmuls are due
                XYs = [emit_E(0), emit_E(1), emit_E(2)]
                emit_F(0, XYs[0])
                XYs.append(emit_E(3))
                emit_F(1, XYs[1])
                emit_F(2, XYs[2])
                emit_F(3, XYs[3])
                if vg == NVG // 2 - 1:
                    # first n-half done: AllReduce it while second half runs
                    # (copy on vector — the scalar queue gates the relu chain)
                    nc.vector.tensor_copy(out=ymA, in_=fps[:, :, 0:D])
                    nc.sync.dma_start(out=ccA_in[:], in_=ymA)
                    nc.gpsimd.collective_compute(
                        "AllReduce", ALU.add, replica_groups=REPL,
                        ins=[ccA_in[:]], outs=[ccA_out[:]],
                    )
                    nc.sync.dma_start(out=ymr, in_=ccA_out[:])

            nc.scalar.copy(out=ymB, in_=fps[:, :, D : 2 * D])
            nc.sync.dma_start(out=ccB_in[:], in_=ymB)
            nc.gpsimd.collective_compute(
                "AllReduce", ALU.add, replica_groups=REPL,
                ins=[ccB_in[:]], outs=[ccB_out[:]],
            )
            nc.scalar.dma_start(out=ymr2, in_=ccB_out[:])
            ymsum = ymr
            nc.vector.tensor_add(ymsum, ymr, ymr2)

            # ============ x = LN(x + LN(yMLP)) ============================
            st1 = statpool.tile([P, TC, 6], FP32, tag="s1", name=f"s1{l}")
            mv1 = statpool.tile([P, TC, 2], FP32, tag="m1", name=f"m1{l}")
            r1 = statpool.tile([P, TC, 1], FP32, tag="r1", name=f"r1{l}")
            for jp in range(TC):
                nc.vector.bn_stats(out=st1[:, jp, :], in_=ymsum[:, jp, :])
                nc.vector.bn_aggr(out=mv1[:, jp, :], in_=st1[:, jp, :])
            nc.scalar.activation(
                out=r1, in_=mv1[:, :, 1:2], func=AF.Sqrt, bias=epst
            )
            nc.vector.reciprocal(r1, r1)
            xmid = mixpool.tile([P, TC, D], FP32, tag="xmid", name=f"xm{l}")
            for jp in range(TC):
                nc.vector.scalar_tensor_tensor(
                    out=xmid[:, jp, :],
                    in0=ymsum[:, jp, :],
                    scalar=r1[:, jp, :],
                    in1=x_f[:, jp, :],
                    op0=ALU.mult,
                    op1=ALU.add,
                )
            st2 = statpool.tile([P, TC, 6], FP32, tag="s2", name=f"s2{l}")
            mv2 = statpool.tile([P, TC, 2], FP32, tag="m2", name=f"m2{l}")
            r2 = statpool.tile([P, TC, 1], FP32, tag="r2", name=f"r2{l}")
            for jp in range(TC):
                nc.vector.bn_stats(out=st2[:, jp, :], in_=xmid[:, jp, :])
                nc.vector.bn_aggr(out=mv2[:, jp, :], in_=st2[:, jp, :])
            nc.scalar.activation(
                out=r2, in_=mv2[:, :, 1:2], func=AF.Sqrt, bias=epst
            )
            nc.vector.reciprocal(r2, r2)
            x_bf_new = xpool.tile([P, TC, D], BF, tag="xbf", name=f"x_bf{l + 1}")
            for jp in range(TC):
                nc.vector.tensor_scalar(
                    out=x_bf_new[:, jp, :],
                    in0=xmid[:, jp, :],
                    scalar1=mv2[:, jp, 0:1],
                    scalar2=r2[:, jp, :],
                    op0=ALU.subtract,
                    op1=ALU.mult,
                )
            x_T_new = xpool.tile([P, DT, T], BF, tag="xT", name=f"x_T{l + 1}")
            for dt_ in range(DT):
                tp = apsum.tile([P, TC, P], BF, tag="quad", name=f"xtp{l}_{dt_}")
                for jp in range(TC):
                    nc.tensor.transpose(
                        tp[:, jp, :], x_bf_new[:, jp, ds(dt_ * P, P)], ident
                    )
                out_ap = x_T_new[:, dt_, :].rearrange("p (a b) -> p a b", a=TC)
                if dt_ == 0:
                    nc.scalar.copy(out=out_ap, in_=tp)
                else:
                    nc.vector.tensor_copy(out=out_ap, in_=tp)
            x_f_new = xpool.tile([P, TC, D], FP32, tag="xf", name=f"x_f{l + 1}")
            for jp in range(TC):
                nc.vector.tensor_scalar(
                    out=x_f_new[:, jp, :],
                    in0=xmid[:, jp, :],
                    scalar1=mv2[:, jp, 0:1],
                    scalar2=r2[:, jp, :],
                    op0=ALU.subtract,
                    op1=ALU.mult,
                )
            return x_f_new, x_bf_new, x_T_new

        for l in range(N_LAYER):
            x_f, x_bf, x_T = emit_layer(l, x_f, x_bf, x_T)

        # ---------------- lm head -----------------------------------------
        lps = cpsum.tile([P, TC, T], FP32, tag="mm", name="lps")
        for jp in range(TC):
            for dt_ in range(DT):
                nc.tensor.matmul(
                    lps[:, jp, :VOCAB],
                    lhsT=x_T[:, dt_, ds(jp * P, P)],
                    rhs=lmt[:, dt_, :],
                    start=(dt_ == 0),
                    stop=(dt_ == DT - 1),
                )
        lout = mixpool.tile([P, TC, VOCAB], FP32, tag="lout", name="lout")
        nc.scalar.copy(out=lout, in_=lps[:, :, :VOCAB])
        nc.sync.dma_start(out=out_d[:], in_=lout)

    if not nc.is_finalized():
        nc.finalize()
    return nc


def _ln_np(x):
    m = x.mean(-1, keepdims=True)
    v = ((x - m) ** 2).mean(-1, keepdims=True)
    return (x - m) / np.sqrt(v + EPS)


def _make_tables():
    t = np.arange(N, dtype=np.float32)
    q = np.floor(t / 2.0) * 2.0
    freqs = (1.0 / (THETA ** (q / N)) / (2.0 * np.float32(math.pi))).astype(
        np.float32
    )
    phases = np.arange(T, dtype=np.float32)[:, None] * freqs[None, :]
    ph = np.float32(np.float32(phases % 1.0) * np.float32(2.0 * math.pi))
    return np.cos(ph).astype(np.float32), np.sin(ph).astype(np.float32)


def _prep_inputs(idx, embed_w, encoder, encoder_v, decoder, lm_head):
    perm = np.concatenate([np.arange(HALF) * 2, np.arange(HALF) * 2 + 1])

    cos, sin = _make_tables()
    # even/odd n share a frequency: only keep the even half, sign folded
    # into subtract (first half) / add (second half) in emit_rope_C.
    cosp = cos[:, perm][:, :HALF]
    sinp = sin[:, perm][:, :HALF]
    # [P, NT2, T]: (p, nt, t) -> table[t, nt*P + p]
    cos_h = np.ascontiguousarray(
        cosp.T.reshape(NT2, P, T).transpose(1, 0, 2)
    ).astype(BF16)
    sin_h = np.ascontiguousarray(
        sinp.T.reshape(NT2, P, T).transpose(1, 0, 2)
    ).astype(BF16)

    mask_h = np.zeros((P, TC, T), np.float32)
    t_idx = np.arange(T)
    for j in range(TC):
        for p in range(P):
            mask_h[p, j] = (t_idx > (j * P + p)).astype(np.float32)
    mask_h = mask_h.astype(BF16)

    lm_h = np.ascontiguousarray(
        lm_head.reshape(DT, P, VOCAB).transpose(1, 0, 2)
    ).astype(BF16)

    x0 = _ln_np(embed_w[idx].astype(np.float32))  # (B, T, D)

    dec3 = decoder.reshape(NH, N, D)

    per_core = []
    for core in range(N_CORES):
        b, h = divmod(core, NH)
        enc_p = encoder[h][:, perm]  # (D, N)
        encv_p = encoder_v[h][:, perm]
        dec_p = dec3[h][perm, :]  # (N, D)

        enc_h = enc_p.reshape(DT, P, NT, P).astype(BF16)
        encv_h = encv_p.reshape(DT, P, NT, P).astype(BF16)
        dec_h = np.ascontiguousarray(
            dec_p.reshape(NT, P, D).transpose(1, 0, 2)
        ).astype(BF16)

        xb = x0[b]  # (T, D) f32
        x0_c = np.ascontiguousarray(
            xb.reshape(TC, P, D).transpose(1, 0, 2)
        ).astype(np.float32)
        x0bf_c = x0_c.astype(BF16)
        x0T_c = np.ascontiguousarray(
            xb.T.reshape(DT, P, T).transpose(1, 0, 2)
        ).astype(BF16)

        per_core.append(
            {
                "x0": x0_c,
                "x0bf": x0bf_c,
                "x0T": x0T_c,
                "enc": enc_h,
                "encv": encv_h,
                "dec": dec_h,
                "cosb": cos_h,
                "sinb": sin_h,
                "maskb": mask_h,
                "lm": lm_h,
            }
        )
    return per_core


def _get_nc():
    if "nc" not in _CACHE:
        _CACHE["nc"] = _build_bass()
    return _CACHE["nc"]


def kernel(idx, embed_w, encoder, encoder_v, decoder, lm_head, **extra):
    idx = np.asarray(idx)
    embed_w = np.asarray(embed_w, dtype=np.float32)
    encoder = np.asarray(encoder, dtype=np.float32)
    encoder_v = np.asarray(encoder_v, dtype=np.float32)
    decoder = np.asarray(decoder, dtype=np.float32)
    lm_head = np.asarray(lm_head, dtype=np.float32)

    nc = _get_nc()
    in_maps = _prep_inputs(idx, embed_w, encoder, encoder_v, decoder, lm_head)
    res = run_bass_kernel_spmd(nc, in_maps, core_ids=list(range(N_CORES)))
    _CACHE["last_results"] = res

    out = np.zeros((B, T, VOCAB), np.float32)
    for b in range(B):
        lg = res.results[b * NH]["logits"]  # [P, TC, VOCAB]
        out[b] = lg.transpose(1, 0, 2).reshape(T, VOCAB)
    return out


if __name__ == "__main__":
    rng = np.random.default_rng(0)
    ins = {
        "idx": rng.integers(0, VOCAB, (B, T)).astype(np.int32),
        "embed_w": (0.02 * rng.standard_normal((VOCAB, D))).astype(np.float32),
        "encoder": (0.02 * rng.standard_normal((NH, D, N))).astype(np.float32),
        "encoder_v": (0.02 * rng.standard_normal((NH, D, N))).astype(np.float32),
        "decoder": (0.02 * rng.standard_normal((NH * N, D))).astype(np.float32),
        "lm_head": (0.02 * rng.standard_normal((D, VOCAB))).astype(np.float32),
    }
    out = kernel(**ins)
    print("out", out.shape, out.dtype, float(np.abs(out).max()))


# revision 47
# speedup vs baseline: 1.0234x; 1.0234x over previous
"""Trainium2 Bass kernel for nn_BDH_39127152067244 (dense_transformer).

Sharding: 8 cores = (b, h) pairs — b = core // 4, h = core % 4. Each core
computes its head's share of every layer; the only cross-core communication
is an AllReduce of the per-head yMLP partial [T, D] once per layer
(replica groups {0..3} and {4..7}), split into two n-halves so the first
AllReduce overlaps the second half of the E/F compute.

Layout tricks:
  - The N axis (8192) is deinterleaved on the host (even n first, odd n
    second). Even/odd pairs share a rope frequency, so the cos/sin tables
    are stored at HALF size ([P, 32, T]) and the rotate-half sign becomes
    subtract (first half) vs add (second half).
  - x_sparse is computed directly in transposed [N, T] layout; the Gram
    matmul (scores) is fused into the A phase: each rope'd group feeds its
    Gram contributions immediately, so the PE never drains between phases.
  - scores: symmetric Gram, strict-lower mask in [t, s] == strict-upper in
    [s, t]; only the needed triangle of tiles is computed.
  - E (y_sparse) and F (decoder matmul) are fused per weight tile; F
    accumulates the two n-halves into separate PSUM half-banks so the first
    half's AllReduce (f32) is fired mid-phase and hidden under compute.
  - All matmuls in bf16 with f32 PSUM accumulation; LayerNorm chains use
    multi-group bn_stats + scalar Rsqrt; residual stream stays f32.
"""

import math
import sys
from contextlib import ExitStack

import numpy as np
import ml_dtypes

sys.path.insert(0, "/opt/trn_rl_repo")

import concourse.bass as bass  # noqa: E402
import concourse.bacc as bacc  # noqa: E402
import concourse.mybir as mybir  # noqa: E402
import concourse.tile as tile  # noqa: E402
from concourse.bass import ds  # noqa: E402
from concourse.bass_utils import run_bass_kernel_spmd  # noqa: E402
from concourse.masks import make_identity  # noqa: E402

BF16 = ml_dtypes.bfloat16
BF = mybir.dt.bfloat16
FP32 = mybir.dt.float32
AF = mybir.ActivationFunctionType
ALU = mybir.AluOpType

# Problem constants (hardcoded per the harness contract).
N_LAYER = 6
D = 256
NH = 4
N = 8192
HALF = N // 2
VOCAB = 256
B, T = 2, 512
THETA = 2.0**16
EPS = 1e-5

P = 128          # partitions
NT = N // P      # 64 n-tiles
NT2 = NT // 2    # 32 table tiles (half-size rope tables)
G4 = 4           # n-tiles per rope/qx group
NG = NT // G4    # 16 groups
VG = 8           # n-tiles per V tile
NVG = NT // VG   # 8 V tiles
TC = T // P      # 4 t-chunks
DT = D // P      # 2 d-tiles
N_CORES = 8
REPL = [[0, 1, 2, 3], [4, 5, 6, 7]]

_CACHE: dict = {}


def _build_bass():
    nc = bacc.Bacc("TRN2", num_devices=N_CORES)

    x0_d = nc.dram_tensor("x0", [P, TC, D], FP32, kind="ExternalInput")
    x0bf_d = nc.dram_tensor("x0bf", [P, TC, D], BF, kind="ExternalInput")
    x0T_d = nc.dram_tensor("x0T", [P, DT, T], BF, kind="ExternalInput")
    enc_d = nc.dram_tensor("enc", [DT, P, NT, P], BF, kind="ExternalInput")
    encv_d = nc.dram_tensor("encv", [DT, P, NT, P], BF, kind="ExternalInput")
    dec_d = nc.dram_tensor("dec", [P, NT, D], BF, kind="ExternalInput")
    cos_d = nc.dram_tensor("cosb", [P, NT2, T], BF, kind="ExternalInput")
    sin_d = nc.dram_tensor("sinb", [P, NT2, T], BF, kind="ExternalInput")
    mask_d = nc.dram_tensor("maskb", [P, TC, T], BF, kind="ExternalInput")
    lm_d = nc.dram_tensor("lm", [P, DT, VOCAB], BF, kind="ExternalInput")
    out_d = nc.dram_tensor("logits", [P, TC, VOCAB], FP32, kind="ExternalOutput")

    with tile.TileContext(nc) as tc, ExitStack() as ctx:
        sb = ctx.enter_context(tc.tile_pool(name="sb", bufs=1))
        vpool = ctx.enter_context(tc.tile_pool(name="vpool", bufs=NVG))
        qrpool = ctx.enter_context(tc.tile_pool(name="qrpool", bufs=3))
        xypool = ctx.enter_context(tc.tile_pool(name="xypool", bufs=4))
        yspool = ctx.enter_context(tc.tile_pool(name="yspool", bufs=3))
        rppool = ctx.enter_context(tc.tile_pool(name="rppool", bufs=2))
        tabpool = ctx.enter_context(tc.tile_pool(name="tabpool", bufs=2))
        wpool = ctx.enter_context(tc.tile_pool(name="wpool", bufs=2))
        mixpool = ctx.enter_context(tc.tile_pool(name="mixpool", bufs=1))
        statpool = ctx.enter_context(tc.tile_pool(name="statpool", bufs=4))
        xpool = ctx.enter_context(tc.tile_pool(name="xpool", bufs=2))
        apsum = ctx.enter_context(tc.tile_pool(name="apsum", bufs=2, space="PSUM"))
        cpsum = ctx.enter_context(tc.tile_pool(name="cpsum", bufs=1, space="PSUM"))
        drm = ctx.enter_context(tc.tile_pool(name="drm", bufs=2, space="DRAM"))

        x_T = xpool.tile([P, DT, T], BF, tag="xT", name="x_T0")
        nc.sync.dma_start(out=x_T, in_=x0T_d[:])
        x_bf = xpool.tile([P, TC, D], BF, tag="xbf", name="x_bf0")
        nc.sync.dma_start(out=x_bf, in_=x0bf_d[:])
        x_f = xpool.tile([P, TC, D], FP32, tag="xf", name="x_f0")
        nc.sync.dma_start(out=x_f, in_=x0_d[:])

        ident = sb.tile([P, P], BF, name="ident")
        make_identity(nc, ident)
        epst = sb.tile([P, 1], FP32, name="epst")
        nc.vector.memset(epst, EPS)
        maskt = sb.tile([P, TC, T], BF, name="maskt")
        nc.sync.dma_start(out=maskt, in_=mask_d[:])
        lmt = sb.tile([P, DT, VOCAB], BF, name="lmt")
        nc.sync.dma_start(out=lmt, in_=lm_d[:])

        # Warm up the collective path (realistic payload size) before
        # layer 0 needs it.
        wsb = sb.tile([P, 512], FP32, name="wsb")
        nc.vector.memset(wsb, 0.0)
        warm_in = drm.tile([P, 512], FP32, tag="win", name="warm_in")
        warm_out = drm.tile([P, 512], FP32, tag="wout", name="warm_out")
        nc.sync.dma_start(out=warm_in[:], in_=wsb)
        nc.gpsimd.collective_compute(
            "AllReduce", ALU.add, replica_groups=REPL,
            ins=[warm_in[:]], outs=[warm_out[:]],
        )

        def emit_layer(l, x_f, x_bf, x_T):
            # ============ fused A + rope + Gram (C) =======================
            V = [None] * NVG
            gps = cpsum.tile([P, TC, T], FP32, tag="mm", name=f"gps{l}")

            def emit_A(vg):
                encg = wpool.tile([P, DT, VG, P], BF, tag="w", name=f"enc{l}_{vg}")
                nc.sync.dma_start(
                    out=encg,
                    in_=enc_d[:, :, ds(vg * VG, VG), :].rearrange(
                        "dt p nt n -> p dt nt n"
                    ),
                )
                vt = vpool.tile([P, VG, T], BF, tag="v", name=f"v{l}_{vg}")
                V[vg] = vt
                for q in range(VG // 2):
                    ps = apsum.tile(
                        [P, 2, T], FP32, tag="quad", name=f"aps{l}_{vg}_{q}"
                    )
                    for i in range(2):
                        for dt_ in range(DT):
                            nc.tensor.matmul(
                                ps[:, i, :],
                                lhsT=encg[:, dt_, q * 2 + i, :],
                                rhs=x_T[:, dt_, :],
                                start=(dt_ == 0),
                                stop=(dt_ == DT - 1),
                            )
                    nc.scalar.activation(
                        out=vt[:, ds(q * 2, 2), :], in_=ps, func=AF.Relu
                    )

            def emit_rope(g, cosg, sing, off4):
                # qr = V[g] * cos -/+ V[g^8] * |sin|  (minus for g<8)
                vg_, off = divmod(g * G4, VG)
                pvg_, poff = divmod((g ^ (NG // 2)) * G4, VG)
                p2 = rppool.tile([P, G4, T], BF, tag="rp2", name=f"rq{l}_{g}")
                nc.vector.tensor_mul(
                    p2, V[pvg_][:, ds(poff, G4), :], sing[:, ds(off4, G4), :]
                )
                qr = qrpool.tile([P, G4, T], BF, tag="qr", name=f"qr{l}_{g}")
                nc.vector.tensor_mul(
                    qr, V[vg_][:, ds(off, G4), :], cosg[:, ds(off4, G4), :]
                )
                if g < NG // 2:
                    nc.vector.tensor_sub(qr, qr, p2)
                else:
                    nc.vector.tensor_add(qr, qr, p2)
                return qr

            def emit_pair_ropes_C(pair):
                cosg = tabpool.tile([P, VG, T], BF, tag="cos", name=f"cos{l}_{pair}")
                nc.sync.dma_start(out=cosg, in_=cos_d[:, ds(pair * VG, VG), :])
                sing = tabpool.tile([P, VG, T], BF, tag="sin", name=f"sin{l}_{pair}")
                nc.sync.dma_start(out=sing, in_=sin_d[:, ds(pair * VG, VG), :])
                gcnt = pair * 4
                for gl in (2 * pair, 2 * pair + 1):
                    off4 = (gl & 1) * G4
                    for g in (gl, gl + NG // 2):
                        qr = emit_rope(g, cosg, sing, off4)
                        for i in range(G4):
                            first = gcnt == 0 and i == 0
                            last = gcnt == NG - 1 and i == G4 - 1
                            for j in range(TC):
                                nc.tensor.matmul(
                                    gps[:, j, : T - j * P],
                                    lhsT=qr[:, i, ds(j * P, P)],
                                    rhs=qr[:, i, ds(j * P, T - j * P)],
                                    start=first,
                                    stop=last,
                                )
                        gcnt += 1

            # C trails A by one pair so the rope chain (vector) has a full
            # extra pair-time to complete before its Gram matmuls are due.
            for pair in range(NVG // 2):
                emit_A(pair)
                emit_A(pair + NVG // 2)
                if pair > 0:
                    emit_pair_ropes_C(pair - 1)
            emit_pair_ropes_C(NVG // 2 - 1)

            # ============ D: yKV = M^T @ x, pipelined LN ==================
            st = mixpool.tile([P, TC, T], BF, tag="st", name=f"st{l}")
            for i in range(TC):
                # only the diagonal 128-block needs masking; the off-diagonal
                # region is strictly upper (mask == 1) — plain copy on scalar
                nc.vector.tensor_mul(
                    st[:, i, ds(i * P, P)],
                    gps[:, i, :P],
                    maskt[:, i, ds(i * P, P)],
                )
                if i < TC - 1:
                    nc.scalar.copy(
                        out=st[:, i, ds((i + 1) * P, T - (i + 1) * P)],
                        in_=gps[:, i, P : T - i * P],
                    )
            dps = cpsum.tile([P, TC, T], FP32, tag="mm", name=f"dps{l}")
            statd = statpool.tile([P, TC, 6], FP32, tag="sd", name=f"sd{l}")
            mvd = statpool.tile([P, TC, 2], FP32, tag="md", name=f"md{l}")
            rsd = statpool.tile([P, TC, 1], FP32, tag="rd", name=f"rd{l}")
            yln = mixpool.tile([P, TC, D], BF, tag="yln", name=f"yln{l}")
            for jp in range(TC):
                for i in range(jp + 1):
                    nc.tensor.matmul(
                        dps[:, jp, :D],
                        lhsT=st[:, i, ds(jp * P, P)],
                        rhs=x_bf[:, i, :],
                        start=(i == 0),
                        stop=(i == jp),
                    )
                nc.vector.bn_stats(out=statd[:, jp, :], in_=dps[:, jp, :D])
                nc.vector.bn_aggr(out=mvd[:, jp, :], in_=statd[:, jp, :])
            nc.scalar.activation(
                out=rsd, in_=mvd[:, :, 1:2], func=AF.Sqrt, bias=epst
            )
            nc.vector.reciprocal(rsd, rsd)
            for jp in range(TC):
                nc.vector.tensor_scalar(
                    out=yln[:, jp, :],
                    in0=dps[:, jp, :D],
                    scalar1=mvd[:, jp, 0:1],
                    scalar2=rsd[:, jp, :],
                    op0=ALU.subtract,
                    op1=ALU.mult,
                )
            ylnT = mixpool.tile([P, DT, T], BF, tag="ylnT", name=f"ylnT{l}")
            for dt_ in range(DT):
                tp = apsum.tile([P, TC, P], BF, tag="quad", name=f"ytp{l}_{dt_}")
                for jp in range(TC):
                    nc.tensor.transpose(
                        tp[:, jp, :], yln[:, jp, ds(dt_ * P, P)], ident
                    )
                out_ap = ylnT[:, dt_, :].rearrange("p (a b) -> p a b", a=TC)
                if dt_ == 0:
                    nc.scalar.copy(out=out_ap, in_=tp)
                else:
                    nc.vector.tensor_copy(out=out_ap, in_=tp)

            # ============ fused E (gated y_sparse) + F (decoder) ==========
            # fps bank m: n-half A partial in cols [0,D), half B in [D,2D)
            fps = cpsum.tile([P, TC, T], FP32, tag="mm", name=f"fps{l}")
            ymA = mixpool.tile([P, TC, D], BF, tag="ymA", name=f"ymA{l}")
            ymB = mixpool.tile([P, TC, D], BF, tag="ymB", name=f"ymB{l}")
            ccA_in = drm.tile([P, TC, D], BF, tag="ccAi", name=f"ccAi{l}")
            ccA_out = drm.tile([P, TC, D], BF, tag="ccAo", name=f"ccAo{l}")
            ccB_in = drm.tile([P, TC, D], BF, tag="ccBi", name=f"ccBi{l}")
            ccB_out = drm.tile([P, TC, D], BF, tag="ccBo", name=f"ccBo{l}")
            ymr = mixpool.tile([P, TC, D], BF, tag="ymr", name=f"ymr{l}")
            ymr2 = mixpool.tile([P, TC, D], BF, tag="ymr2", name=f"ym2{l}")

            for vg in range(NVG):
                evg = wpool.tile([P, DT, VG, P], BF, tag="w", name=f"ev{l}_{vg}")
                nc.sync.dma_start(
                    out=evg,
                    in_=encv_d[:, :, ds(vg * VG, VG), :].rearrange(
                        "dt p nt n -> p dt nt n"
                    ),
                )
                decg = wpool.tile([P, VG, D], BF, tag="dec", name=f"dec{l}_{vg}")
                nc.sync.dma_start(out=decg, in_=dec_d[:, ds(vg * VG, VG), :])
                half = vg // (NVG // 2)
                vgh = vg % (NVG // 2)

                def emit_E(q):
                    ps = apsum.tile(
                        [P, 2, T], FP32, tag="quad", name=f"eps{l}_{vg}_{q}"
                    )
                    for i in range(2):
                        for dt_ in range(DT):
                            nc.tensor.matmul(
                                ps[:, i, :],
                                lhsT=evg[:, dt_, q * 2 + i, :],
                                rhs=ylnT[:, dt_, :],
                                start=(dt_ == 0),
                                stop=(dt_ == DT - 1),
                            )
                    ys = yspool.tile([P, 2, T], BF, tag="ys", name=f"ys{l}_{vg}_{q}")
                    nc.scalar.activation(out=ys, in_=ps, func=AF.Relu)
                    xy = xypool.tile([P, 2, T], BF, tag="xy", name=f"xy{l}_{vg}_{q}")
                    nc.vector.tensor_mul(xy, ys, V[vg][:, ds(q * 2, 2), :])
                    return xy

                def emit_F(q, xy):
                    for i in range(2):
                        kh = vgh * VG + q * 2 + i
                        for m in range(TC):
                            nc.tensor.matmul(
                                fps[:, m, ds(half * D, D)],
                                lhsT=xy[:, i, ds(m * P, P)],
                                rhs=decg[:, q * 2 + i, :],
                                start=(kh == 0),
                                stop=(kh == NT // 2 - 1),
                            )

                # one-quad software pipeline: F trails E so the relu+gate
                # chain of quad q completes before its F matmuls are due
                XYs = [emit_E(0), emit_E(1), emit_E(2)]
                emit_F(0, XYs[0])
                XYs.append(emit_E(3))
                emit_F(1, XYs[1])
                emit_F(2, XYs[2])
                emit_F(3, XYs[3])
                if vg == NVG // 2 - 1:
                    # first n-half done: AllReduce it while second half runs
                    # (copy on vector — the scalar queue gates the relu chain)
                    nc.vector.tensor_copy(out=ymA, in_=fps[:, :, 0:D])
                    nc.sync.dma_start(out=ccA_in[:], in_=ymA)
                    nc.gpsimd.collective_compute(
                        "AllReduce", ALU.add, replica_groups=REPL,
                        ins=[ccA_in[:]], outs=[ccA_out[:]],
                    )
                    nc.sync.dma_start(out=ymr, in_=ccA_out[:])

            nc.scalar.copy(out=ymB, in_=fps[:, :, D : 2 * D])
            nc.sync.dma_start(out=ccB_in[:], in_=ymB)
            nc.gpsimd.collective_compute(
                "AllReduce", ALU.add, replica_groups=REPL,
                ins=[ccB_in[:]], outs=[ccB_out[:]],
            )
            nc.sync.dma_start(out=ymr2, in_=ccB_out[:])
            ymsum = ymr
            nc.vector.tensor_add(ymsum, ymr, ymr2)

            # ============ x = LN(x + LN(yMLP)) ============================
            st1 = statpool.tile([P, TC, 6], FP32, tag="s1", name=f"s1{l}")
            mv1 = statpool.tile([P, TC, 2], FP32, tag="m1", name=f"m1{l}")
            r1 = statpool.tile([P, TC, 1], FP32, tag="r1", name=f"r1{l}")
            for jp in range(TC):
                nc.vector.bn_stats(out=st1[:, jp, :], in_=ymsum[:, jp, :])
                nc.vector.bn_aggr(out=mv1[:, jp, :], in_=st1[:, jp, :])
            nc.scalar.activation(
                out=r1, in_=mv1[:, :, 1:2], func=AF.Sqrt, bias=epst
            )
            nc.vector.reciprocal(r1, r1)
            xmid = mixpool.tile([P, TC, D], FP32, tag="xmid", name=f"xm{l}")
            for jp in range(TC):
                nc.vector.scalar_tensor_tensor(
                    out=xmid[:, jp, :],
                    in0=ymsum[:, jp, :],
                    scalar=r1[:, jp, :],
                    in1=x_f[:, jp, :],
                    op0=ALU.mult,
                    op1=ALU.add,
                )
            st2 = statpool.tile([P, TC, 6], FP32, tag="s2", name=f"s2{l}")
            mv2 = statpool.tile([P, TC, 2], FP32, tag="m2", name=f"m2{l}")
            r2 = statpool.tile([P, TC, 1], FP32, tag="r2", name=f"r2{l}")
            for jp in range(TC):
                nc.vector.bn_stats(out=st2[:, jp, :], in_=xmid[:, jp, :])
                nc.vector.bn_aggr(out=mv2[:, jp, :], in_=st2[:, jp, :])
            nc.scalar.activation(
                out=r2, in_=mv2[:, :, 1:2], func=AF.Sqrt, bias=epst
            )
            nc.vector.reciprocal(r2, r2)
            x_bf_new = xpool.tile([P, TC, D], BF, tag="xbf", name=f"x_bf{l + 1}")
            for jp in range(TC):
                nc.vector.tensor_scalar(
                    out=x_bf_new[:, jp, :],
                    in0=xmid[:, jp, :],
                    scalar1=mv2[:, jp, 0:1],
                    scalar2=r2[:, jp, :],
                    op0=ALU.subtract,
                    op1=ALU.mult,
                )
            x_T_new = xpool.tile([P, DT, T], BF, tag="xT", name=f"x_T{l + 1}")
            for dt_ in range(DT):
                tp = apsum.tile([P, TC, P], BF, tag="quad", name=f"xtp{l}_{dt_}")
                for jp in range(TC):
                    nc.tensor.transpose(
                        tp[:, jp, :], x_bf_new[:, jp, ds(dt_ * P, P)], ident
                    )
                out_ap = x_T_new[:, dt_, :].rearrange("p (a b) -> p a b", a=TC)
                if dt_ == 0:
                    nc.scalar.copy(out=out_ap, in_=tp)
                else:
                    nc.vector.tensor_copy(out=out_ap, in_=tp)
            x_f_new = xpool.tile([P, TC, D], FP32, tag="xf", name=f"x_f{l + 1}")
            for jp in range(TC):
                nc.vector.tensor_scalar(
                    out=x_f_new[:, jp, :],
                    in0=xmid[:, jp, :],
                    scalar1=mv2[:, jp, 0:1],
                    scalar2=r2[:, jp, :],
                    op0=ALU.subtract,
                    op1=ALU.mult,
                )
            return x_f_new, x_bf_new, x_T_new

        for l in range(N_LAYER):
            x_f, x_bf, x_T = emit_layer(l, x_f, x_bf, x_T)

        # ---------------- lm head -----------------------------------------
        lps = cpsum.tile([P, TC, T], FP32, tag="mm", name="lps")
        for jp in range(TC):
            for dt_ in range(DT):
                nc.tensor.matmul(
                    lps[:, jp, :VOCAB],
                    lhsT=x_T[:, dt_, ds(jp * P, P)],
                    rhs=lmt[:, dt_, :],
                    start=(dt_ == 0),
                    stop=(dt_ == DT - 1),
                )
        lout = mixpool.tile([P, TC, VOCAB], FP32, tag="lout", name="lout")
        nc.scalar.copy(out=lout, in_=lps[:, :, :VOCAB])
        nc.sync.dma_start(out=out_d[:], in_=lout)

    if not nc.is_finalized():
        nc.finalize()
    return nc


def _ln_np(x):
    m = x.mean(-1, keepdims=True)
    v = ((x - m) ** 2).mean(-1, keepdims=True)
    return (x - m) / np.sqrt(v + EPS)


def _make_tables():
    t = np.arange(N, dtype=np.float32)
    q = np.floor(t / 2.0) * 2.0
    freqs = (1.0 / (THETA ** (q / N)) / (2.0 * np.float32(math.pi))).astype(
        np.float32
    )
    phases = np.arange(T, dtype=np.float32)[:, None] * freqs[None, :]
    ph = np.float32(np.float32(phases % 1.0) * np.float32(2.0 * math.pi))
    return np.cos(ph).astype(np.float32), np.sin(ph).astype(np.float32)


def _prep_inputs(idx, embed_w, encoder, encoder_v, decoder, lm_head):
    perm = np.concatenate([np.arange(HALF) * 2, np.arange(HALF) * 2 + 1])

    cos, sin = _make_tables()
    # even/odd n share a frequency: only keep the even half, sign folded
    # into subtract (first half) / add (second half) in emit_rope_C.
    cosp = cos[:, perm][:, :HALF]
    sinp = sin[:, perm][:, :HALF]
    # [P, NT2, T]: (p, nt, t) -> table[t, nt*P + p]
    cos_h = np.ascontiguousarray(
        cosp.T.reshape(NT2, P, T).transpose(1, 0, 2)
    ).astype(BF16)
    sin_h = np.ascontiguousarray(
        sinp.T.reshape(NT2, P, T).transpose(1, 0, 2)
    ).astype(BF16)

    mask_h = np.zeros((P, TC, T), np.float32)
    t_idx = np.arange(T)
    for j in range(TC):
        for p in range(P):
            mask_h[p, j] = (t_idx > (j * P + p)).astype(np.float32)
    mask_h = mask_h.astype(BF16)

    lm_h = np.ascontiguousarray(
        lm_head.reshape(DT, P, VOCAB).transpose(1, 0, 2)
    ).astype(BF16)

    x0 = _ln_np(embed_w[idx].astype(np.float32))  # (B, T, D)

    dec3 = decoder.reshape(NH, N, D)

    per_core = []
    for core in range(N_CORES):
        b, h = divmod(core, NH)
        enc_p = encoder[h][:, perm]  # (D, N)
        encv_p = encoder_v[h][:, perm]
        dec_p = dec3[h][perm, :]  # (N, D)

        enc_h = enc_p.reshape(DT, P, NT, P).astype(BF16)
        encv_h = encv_p.reshape(DT, P, NT, P).astype(BF16)
        dec_h = np.ascontiguousarray(
            dec_p.reshape(NT, P, D).transpose(1, 0, 2)
        ).astype(BF16)

        xb = x0[b]  # (T, D) f32
        x0_c = np.ascontiguousarray(
            xb.reshape(TC, P, D).transpose(1, 0, 2)
        ).astype(np.float32)
        x0bf_c = x0_c.astype(BF16)
        x0T_c = np.ascontiguousarray(
            xb.T.reshape(DT, P, T).transpose(1, 0, 2)
        ).astype(BF16)

        per_core.append(
            {
                "x0": x0_c,
                "x0bf": x0bf_c,
                "x0T": x0T_c,
                "enc": enc_h,
                "encv": encv_h,
                "dec": dec_h,
                "cosb": cos_h,
                "sinb": sin_h,
                "maskb": mask_h,
                "lm": lm_h,
            }
        )
    return per_core


def _get_nc():
    if "nc" not in _CACHE:
        _CACHE["nc"] = _build_bass()
    return _CACHE["nc"]


def kernel(idx, embed_w, encoder, encoder_v, decoder, lm_head, **extra):
    idx = np.asarray(idx)
    embed_w = np.asarray(embed_w, dtype=np.float32)
    encoder = np.asarray(encoder, dtype=np.float32)
    encoder_v = np.asarray(encoder_v, dtype=np.float32)
    decoder = np.asarray(decoder, dtype=np.float32)
    lm_head = np.asarray(lm_head, dtype=np.float32)

    nc = _get_nc()
    in_maps = _prep_inputs(idx, embed_w, encoder, encoder_v, decoder, lm_head)
    res = run_bass_kernel_spmd(nc, in_maps, core_ids=list(range(N_CORES)))
    _CACHE["last_results"] = res

    out = np.zeros((B, T, VOCAB), np.float32)
    for b in range(B):
        lg = res.results[b * NH]["logits"]  # [P, TC, VOCAB]
        out[b] = lg.transpose(1, 0, 2).reshape(T, VOCAB)
    return out


if __name__ == "__main__":
    rng = np.random.default_rng(0)
    ins = {
        "idx": rng.integers(0, VOCAB, (B, T)).astype(np.int32),
        "embed_w": (0.02 * rng.standard_normal((VOCAB, D))).astype(np.float32),
        "encoder": (0.02 * rng.standard_normal((NH, D, N))).astype(np.float32),
        "encoder_v": (0.02 * rng.standard_normal((NH, D, N))).astype(np.float32),
        "decoder": (0.02 * rng.standard_normal((NH * N, D))).astype(np.float32),
        "lm_head": (0.02 * rng.standard_normal((D, VOCAB))).astype(np.float32),
    }
    out = kernel(**ins)
    print("out", out.shape, out.dtype, float(np.abs(out).max()))
